# revision 10
# baseline (speedup 1.0000x reference)
"""GroupLinear (soft MoE routing) Trainium2 Bass kernel.

Computes out[b,o] = sum_j g[b,j] * (x[b,:] @ W[j,:,:])[o] + (g @ bias_p)[b,o]
for B=16384, G=16, DIN=DOUT=512, fp32.

Sharding: data-parallel over batch across 8 NeuronCores (2048 rows/core);
weight + bias replicated.

Per-core schedule (PE-roofline oriented; PE stream is the critical path at
~231 ns per 512-row fp32r matmul):
  - W resident in SBUF, DMA'd as float32r on the SP HWDGE queue; x/g/bias
    startup loads go on the Activation HWDGE queue so W[0] and x0 arrive
    concurrently (~2.5us earlier than a single-queue issue).
  - fp32r warmup matmuls fill the PE from the end of the framework preamble
    until W[0] lands, ramping the HAM clock toward 8/8.
  - phase A (tiles 0-7): GROUP loop outermost so per-W-slice compute
    (~7.4us) stays above the W DMA arrival rate; x transposes (fp32r,
    1.5 cy/row) + gT + bias-seed matmuls emitted per-tile inside the j=0
    sweep. acc chain: scale on ScalarE/VectorE, adds on VectorE/GpSimdE.
  - phase B (tiles 8-15): TILE loop outermost (W fully resident by then):
    each tile runs its 16 groups back-to-back, finishing staggered, and its
    output DMA streams out immediately — no end-of-kernel DMA burst. Scales
    on ScalarE, 16-add chain on VectorE (8.5us < 15.3us tile period).
"""

import numpy as np

import concourse.bass as bass
import concourse.tile as tile
from concourse import bacc, mybir
from concourse.bass_utils import run_bass_kernel_spmd
from concourse.masks import make_identity

B, G, DIN, DOUT = 16384, 16, 512, 512
NCORES = 8
BC = B // NCORES          # rows per core
P = 128                   # partitions
NBT = BC // P             # batch tiles per core (16)
KC = DIN // P             # contraction chunks (4)
PB = 8                    # batch tiles per phase
NPH = NBT // PB           # phases (2)

F32 = mybir.dt.float32
F32R = mybir.dt.float32r

# phase A per phase-slot k: which engine scales Y and which accumulates.
# GpSimd (slowest) gets the earliest slots so sweeps drain on fast engines;
# ScalarE scales the slots whose adds are on DVE/GpSimd.
SCALE_ON_ACT = {2, 3, 4, 5, 6, 7}   # else VectorE
CHAIN_ON_DVE = {3, 4, 5, 6, 7}      # else GpSimdE


def _emit(nc, tc, out_ap, x_ap, g_ap, w_ap, bias_ap, ctx):
    const_pool = ctx.enter_context(tc.tile_pool(name="const", bufs=1))
    wpool = ctx.enter_context(tc.tile_pool(name="wpool", bufs=1))
    xpool = ctx.enter_context(tc.tile_pool(name="xpool", bufs=6))
    gpool = ctx.enter_context(tc.tile_pool(name="gpool", bufs=2 * PB + 2))
    xtpool = ctx.enter_context(tc.tile_pool(name="xtpool", bufs=PB + 1))
    gtpool = ctx.enter_context(tc.tile_pool(name="gtpool", bufs=PB + 1))
    accpool = ctx.enter_context(tc.tile_pool(name="accpool", bufs=PB + 1))
    tmppool = ctx.enter_context(tc.tile_pool(name="tmppool", bufs=5))
    ps_y = ctx.enter_context(tc.tile_pool(name="ps_y", bufs=4, space="PSUM"))
    ps_yb = ctx.enter_context(tc.tile_pool(name="ps_yb", bufs=2, space="PSUM"))
    ps_t = ctx.enter_context(tc.tile_pool(name="ps_t", bufs=2, space="PSUM"))

    # fp32r identity: transpose cost is keyed on the moving operand (the
    # identity), and fp32r streams 1.5 cy/row vs 2.0 for fp32. Built as
    # fp32 (gpsimd memset/affine_select reject f32r) then rounded into an
    # f32r tile via ScalarE copy, which satisfies the BIR verifier's
    # rounded-to-FP32r requirement. 0/1 are exact under the rounding.
    ident32 = const_pool.tile([P, P], F32, name="ident32")
    make_identity(nc, ident32)
    identr = const_pool.tile([P, P], F32R, name="identr")
    nc.scalar.copy(identr[:], ident32[:])
    ident = identr[:]

    # PE warmup: dependency-free fp32r matmuls from the end of the framework
    # preamble until W[0]/x0 arrive, so the HAM clock gate ramps toward 8/8
    # before the real matmul stream begins.
    dum = const_pool.tile([P, DOUT], F32, name="dum")
    nc.gpsimd.memset(dum[:], 1.0)
    for wi in range(4):
        wps = ps_t.tile([P, DOUT], F32, tag="tps", name="wps")
        nc.tensor.matmul(wps[:], dum[:, 0:P], dum[:], start=True, stop=True)

    def issue_load(bt, eng):
        xt = xpool.tile([P, DIN], F32R, tag="xt", name=f"xt{bt}")
        eng.dma_start(xt[:], x_ap[bt * P:(bt + 1) * P, :])
        gt = gpool.tile([P, G], F32, tag="gt", name=f"gt{bt}")
        eng.dma_start(gt[:], g_ap[bt * P:(bt + 1) * P, :])
        return xt, gt

    def issue_transpose_x(xt, bt):
        """PE transposes of x (fp32r, 1.5 cy/row); PSUM->SBUF copies on ScalarE."""
        xT = xtpool.tile([P, DIN], F32R, tag="xT", name=f"xT{bt}")
        for ic in range(KC):
            tps = ps_t.tile([P, P], F32R, tag="tps", name="tps")
            nc.tensor.transpose(tps[:], xt[:, ic * P:(ic + 1) * P], ident)
            nc.scalar.copy(xT[:, ic * P:(ic + 1) * P], tps[:])
        return xT

    def issue_gt(gt, bt):
        gps = ps_t.tile([G, P], F32, tag="tps", name="gps")
        nc.tensor.transpose(gps[:], gt[:], ident32[:])
        gT = gtpool.tile([G, P], F32R, tag="gT", name=f"gT{bt}")
        nc.scalar.copy(gT[:], gps[:])
        return gT

    w_sb = wpool.tile([P, G * KC * DOUT], F32R, name="w_sb")

    def issue_w(j):
        for ic in range(KC):
            nc.sync.dma_start(
                w_sb[:, (j * KC + ic) * DOUT:(j * KC + ic + 1) * DOUT],
                w_ap[j, ic * P:(ic + 1) * P, :],
            )

    # startup: W stream on the SP queue, x/g/bias on the Activation queue so
    # W[0] and the first x tiles transfer concurrently.
    issue_w(0)
    loads = {0: issue_load(0, nc.scalar)}
    bias_sb = const_pool.tile([G, DOUT], F32R, name="bias_sb")
    nc.scalar.dma_start(bias_sb[:], bias_ap[:, :])
    issue_w(1)
    for bt in range(1, PB):
        loads[bt] = issue_load(bt, nc.scalar)
    for j in range(2, G):
        issue_w(j)
    # phase B x/g loads: SP queue, transfers land ~60us, needed at ~140us.
    for bt in range(PB, NBT):
        loads[bt] = issue_load(bt, nc.sync)

    def matmul_y(xT, j):
        y = ps_y.tile([P, DOUT], F32, tag="y", name="y")
        for ic in range(KC):
            nc.tensor.matmul(
                y[:],
                xT[:, ic * P:(ic + 1) * P],
                w_sb[:, (j * KC + ic) * DOUT:(j * KC + ic + 1) * DOUT],
                start=(ic == 0),
                stop=(ic == KC - 1),
            )
        return y

    # ---- phase A: tiles 0..7, group loop outermost (paced by W arrival) ----
    trs = {}
    gts = {}
    accs = {}
    for j in range(G):
        for k, bt in enumerate(range(PB)):
            gt = loads[bt][1]
            if j == 0:
                # JIT per-tile prep inside the j=0 sweep: x transposes, gT,
                # and the bias-seed matmul yb = gT @ bias_p.
                trs[bt] = issue_transpose_x(loads[bt][0], bt)
                gts[bt] = issue_gt(gt, bt)
            y = matmul_y(trs[bt], j)
            if j == 0:
                yb = ps_yb.tile([P, DOUT], F32, tag="yb", name=f"yb{bt}")
                nc.tensor.matmul(yb[:], gts[bt][:], bias_sb[:], start=True, stop=True)
            tmp = tmppool.tile([P, DOUT], F32, tag="tmp", name=f"tmp{k}")
            if k in SCALE_ON_ACT:
                nc.scalar.mul(tmp[:], y[:], gt[:, j:j + 1])
            else:
                nc.vector.tensor_scalar_mul(tmp[:], y[:], gt[:, j:j + 1])

            if j == 0:
                acc = accpool.tile([P, DOUT], F32, tag="acc", name=f"acc{bt}")
                nc.vector.tensor_add(acc[:], yb[:], tmp[:])
                accs[bt] = acc
            elif k in CHAIN_ON_DVE:
                nc.vector.tensor_add(accs[bt][:], accs[bt][:], tmp[:])
            else:
                nc.gpsimd.tensor_add(accs[bt][:], accs[bt][:], tmp[:])

    for bt in range(PB):
        nc.sync.dma_start(out_ap[bt * P:(bt + 1) * P, :], accs[bt][:])

    # ---- phase B: tiles 8..15, tile loop outermost (W fully resident) ----
    # Each tile finishes all 16 groups then streams its output immediately,
    # so the kernel tail is one tile's drain instead of eight.
    for bt in range(PB, NBT):
        xt, gt = loads[bt]
        xT = issue_transpose_x(xt, bt)
        gT = issue_gt(gt, bt)
        yb = ps_yb.tile([P, DOUT], F32, tag="yb", name=f"yb{bt}")
        nc.tensor.matmul(yb[:], gT[:], bias_sb[:], start=True, stop=True)
        acc = accpool.tile([P, DOUT], F32, tag="acc", name=f"acc{bt}")
        for j in range(G):
            y = matmul_y(xT, j)
            tmp = tmppool.tile([P, DOUT], F32, tag="tmp", name=f"tmp{j % 4}")
            nc.scalar.mul(tmp[:], y[:], gt[:, j:j + 1])
            if j == 0:
                nc.vector.tensor_add(acc[:], yb[:], tmp[:])
            else:
                nc.vector.tensor_add(acc[:], acc[:], tmp[:])
        nc.sync.dma_start(out_ap[bt * P:(bt + 1) * P, :], acc[:])


def _build():
    nc = bacc.Bacc("TRN2", target_bir_lowering=False, debug=False)
    # x/weight/bias declared float32r (same 4-byte layout as fp32 on the
    # host) so DMA feeds the FP32r matmuls/transposes with no conversion
    x_ap = nc.dram_tensor("x", [BC, DIN], F32R, kind="ExternalInput").ap()
    g_ap = nc.dram_tensor("g", [BC, G], F32, kind="ExternalInput").ap()
    w_ap = nc.dram_tensor("weight", [G, DIN, DOUT], F32R, kind="ExternalInput").ap()
    bias_ap = nc.dram_tensor("bias_p", [G, DOUT], F32R, kind="ExternalInput").ap()
    out_ap = nc.dram_tensor("out", [BC, DOUT], F32, kind="ExternalOutput").ap()

    from contextlib import ExitStack

    with tile.TileContext(nc) as tc:
        with ExitStack() as ctx:
            _emit(nc, tc, out_ap, x_ap, g_ap, w_ap, bias_ap, ctx)
    nc.compile()
    return nc


_NC = None
last_result = None


def kernel(x, g, weight, bias_p):
    global _NC, last_result
    if _NC is None:
        _NC = _build()

    x = np.ascontiguousarray(np.asarray(x, dtype=np.float32))
    g = np.ascontiguousarray(np.asarray(g, dtype=np.float32))
    weight = np.ascontiguousarray(np.asarray(weight, dtype=np.float32))
    bias_p = np.ascontiguousarray(np.asarray(bias_p, dtype=np.float32))

    in_maps = [
        {
            "x": x[c * BC:(c + 1) * BC],
            "g": g[c * BC:(c + 1) * BC],
            "weight": weight,
            "bias_p": bias_p,
        }
        for c in range(NCORES)
    ]
    res = run_bass_kernel_spmd(_NC, in_maps, core_ids=list(range(NCORES)))
    last_result = res
    return np.concatenate([r["out"] for r in res.results], axis=0)


# revision 17
# speedup vs baseline: 1.0876x; 1.0876x over previous
"""GroupLinear (soft MoE routing) Trainium2 Bass kernel.

Computes out[b,o] = sum_j g[b,j] * (x[b,:] @ W[j,:,:])[o] + (g @ bias_p)[b,o]
for B=16384, G=16, DIN=DOUT=512, fp32.

Sharding: data-parallel over batch across 8 NeuronCores (2048 rows/core);
weight + bias replicated.

Per-core schedule (PE-roofline oriented; the PE stream is the critical path
at ~231 ns per 512-row fp32r matmul, and any multi-us PE stall also drops
the HAM clock to 4/8 for tens of us — so every engine that gates the PE
must stay far ahead):
  - W streams on the SP HWDGE queue; x0/g/bias startup loads on the
    Activation queue so W[0] and x0 transfer concurrently. g tiles for a
    whole phase load as ONE batched DMA ([128, 8, 16]).
  - fp32 warmup matmuls cover the framework preamble -> W[0] arrival window
    and start the HAM clock ramp.
  - group-mix accumulation uses the fused VectorE scalar_tensor_tensor:
    acc = y * g[:,j] + acc (one op per (j,tile) instead of scale+add, with
    the j=0 op seeding from the bias matmul: acc = y*g0 + yb). VectorE is
    the only PSUM drain for y (8.5us per 16-group chain vs 15.1us of PE per
    tile); ScalarE only does the small transpose copies, so neither can
    back-pressure the PE's PSUM rotation.
  - phase A (tiles 0-7): group loop outermost, paced by W arrival; x
    transposes (fp32r identity, 1.5 cy/row) + gT + bias matmul emitted
    per-tile inside the j=0 sweep.
  - phase B (tiles 8-15): tile loop outermost (W resident); each tile's
    output DMA streams out as soon as its chain ends - no tail burst.
"""

import numpy as np

import concourse.bass as bass
import concourse.tile as tile
from concourse import bacc, mybir
from concourse.bass_utils import run_bass_kernel_spmd
from concourse.masks import make_identity

B, G, DIN, DOUT = 16384, 16, 512, 512
NCORES = 8
BC = B // NCORES          # rows per core
P = 128                   # partitions
NBT = BC // P             # batch tiles per core (16)
KC = DIN // P             # contraction chunks (4)
PB = 8                    # batch tiles per phase
NPH = NBT // PB           # phases (2)

F32 = mybir.dt.float32
F32R = mybir.dt.float32r
MULT = mybir.AluOpType.mult
ADD = mybir.AluOpType.add


def _emit(nc, tc, out_ap, x_ap, g_ap, w_ap, bias_ap, ctx):
    const_pool = ctx.enter_context(tc.tile_pool(name="const", bufs=1))
    wpool = ctx.enter_context(tc.tile_pool(name="wpool", bufs=1))
    xpool = ctx.enter_context(tc.tile_pool(name="xpool", bufs=6))
    gpool = ctx.enter_context(tc.tile_pool(name="gpool", bufs=2))
    xtpool = ctx.enter_context(tc.tile_pool(name="xtpool", bufs=PB + 1))
    gtpool = ctx.enter_context(tc.tile_pool(name="gtpool", bufs=PB + 1))
    accpool = ctx.enter_context(tc.tile_pool(name="accpool", bufs=PB + 1))
    ybspool = ctx.enter_context(tc.tile_pool(name="ybspool", bufs=3))
    ps_y = ctx.enter_context(tc.tile_pool(name="ps_y", bufs=4, space="PSUM"))
    ps_yb = ctx.enter_context(tc.tile_pool(name="ps_yb", bufs=2, space="PSUM"))
    ps_t = ctx.enter_context(tc.tile_pool(name="ps_t", bufs=2, space="PSUM"))

    # fp32r identity: transpose cost is keyed on the moving operand (the
    # identity); fp32r streams 1.5 cy/row vs 2.0 for fp32. Built as fp32
    # (gpsimd memset/affine_select reject f32r), rounded into an f32r tile
    # via ScalarE copy (satisfies the rounded-to-FP32r BIR check; 0/1 are
    # exact). A plain fp32 identity serves the fp32 g transposes.
    ident32 = const_pool.tile([P, P], F32, name="ident32")
    make_identity(nc, ident32)
    identr = const_pool.tile([P, P], F32R, name="identr")
    nc.scalar.copy(identr[:], ident32[:])

    # startup loads: Activation HWDGE queue (frees SP for the W stream).
    # ScalarE is drain-idle until the first transposes, ~2us after these.
    xpA = xpool.tile([P, DIN], F32R, tag="xt", name="xt0")
    nc.scalar.dma_start(xpA[:], x_ap[0:P, :])
    gA = gpool.tile([P, PB, G], F32, tag="g", name="gA")
    nc.scalar.dma_start(
        gA[:], g_ap[0:PB * P, :].rearrange("(k p) j -> p k j", p=P)
    )
    bias_sb = const_pool.tile([G, DOUT], F32R, name="bias_sb")
    nc.scalar.dma_start(bias_sb[:], bias_ap[:, :])

    # PE warmup: dependency-free matmuls covering preamble -> W[0] arrival,
    # ramping the HAM clock before the real stream begins.
    dum = const_pool.tile([P, DOUT], F32, name="dum")
    nc.gpsimd.memset(dum[:], 1.0)
    for wi in range(3):
        wps = ps_t.tile([P, DOUT], F32, tag="tps", name="wps")
        nc.tensor.matmul(wps[:], dum[:, 0:P], dum[:], start=True, stop=True)

    w_sb = wpool.tile([P, G * KC * DOUT], F32R, name="w_sb")

    def issue_w(j):
        for ic in range(KC):
            nc.sync.dma_start(
                w_sb[:, (j * KC + ic) * DOUT:(j * KC + ic + 1) * DOUT],
                w_ap[j, ic * P:(ic + 1) * P, :],
            )

    def issue_x(bt):
        xt = xpool.tile([P, DIN], F32R, tag="xt", name=f"xt{bt}")
        nc.sync.dma_start(xt[:], x_ap[bt * P:(bt + 1) * P, :])
        return xt

    # SP queue order tuned so everything lands just ahead of first use:
    # W0 (needed ~10.5us), phase-A x tiles, then the W body, then phase B.
    xts = {0: xpA}
    issue_w(0)
    for bt in range(1, 5):
        xts[bt] = issue_x(bt)
    issue_w(1)
    for bt in range(5, PB):
        xts[bt] = issue_x(bt)
    for j in range(2, G):
        issue_w(j)
    for bt in range(PB, NBT):
        xts[bt] = issue_x(bt)
    gB = gpool.tile([P, PB, G], F32, tag="g", name="gB")
    nc.sync.dma_start(
        gB[:], g_ap[PB * P:2 * PB * P, :].rearrange("(k p) j -> p k j", p=P)
    )

    def issue_transpose_x(xt, bt):
        """PE transposes of x (fp32r); PSUM->SBUF copies on ScalarE."""
        xT = xtpool.tile([P, DIN], F32R, tag="xT", name=f"xT{bt}")
        for ic in range(KC):
            tps = ps_t.tile([P, P], F32R, tag="tps", name="tps")
            nc.tensor.transpose(tps[:], xt[:, ic * P:(ic + 1) * P], identr[:])
            nc.scalar.copy(xT[:, ic * P:(ic + 1) * P], tps[:])
        return xT

    def issue_gt(gsrc, bt):
        gps = ps_t.tile([G, P], F32, tag="tps", name="gps")
        nc.tensor.transpose(gps[:], gsrc, ident32[:])
        gT = gtpool.tile([G, P], F32R, tag="gT", name=f"gT{bt}")
        nc.scalar.copy(gT[:], gps[:])
        return gT

    def matmul_y(xT, j):
        y = ps_y.tile([P, DOUT], F32, tag="y", name="y")
        for ic in range(KC):
            nc.tensor.matmul(
                y[:],
                xT[:, ic * P:(ic + 1) * P],
                w_sb[:, (j * KC + ic) * DOUT:(j * KC + ic + 1) * DOUT],
                start=(ic == 0),
                stop=(ic == KC - 1),
            )
        return y

    def fused_step(acc, y, gcol, seed=None):
        # acc = y * g[:,j] + (yb at j=0 else acc): one VectorE op drains the
        # y PSUM bank, applies the group weight, and accumulates.
        nc.vector.scalar_tensor_tensor(
            acc[:], y[:], gcol, (seed if seed is not None else acc)[:], MULT, ADD
        )

    # ---- phase A: tiles 0..7, group loop outermost (paced by W arrival) ----
    trs = {}
    accs = {}
    for j in range(G):
        for k in range(PB):
            if j == 0:
                trs[k] = issue_transpose_x(xts[k], k)
            y = matmul_y(trs[k], j)
            gcol = gA[:, k, j:j + 1]
            if j == 0:
                gT = issue_gt(gA[:, k, :], k)
                yb = ps_yb.tile([P, DOUT], F32, tag="yb", name=f"yb{k}")
                nc.tensor.matmul(yb[:], gT[:], bias_sb[:], start=True, stop=True)
                # the fused op may read only one PSUM operand; stage the
                # bias term through SBUF on the (otherwise idle) ScalarE
                ybs = ybspool.tile([P, DOUT], F32, tag="ybs", name=f"ybs{k}")
                nc.scalar.copy(ybs[:], yb[:])
                acc = accpool.tile([P, DOUT], F32, tag="acc", name=f"acc{k}")
                accs[k] = acc
                fused_step(acc, y, gcol, seed=ybs)
            else:
                fused_step(accs[k], y, gcol)

    for k in range(PB):
        nc.sync.dma_start(out_ap[k * P:(k + 1) * P, :], accs[k][:])

    # ---- phase B: tiles 8..15, tile loop outermost (W fully resident) ----
    for bt in range(PB, NBT):
        k = bt - PB
        xT = issue_transpose_x(xts[bt], bt)
        gT = issue_gt(gB[:, k, :], bt)
        yb = ps_yb.tile([P, DOUT], F32, tag="yb", name=f"yb{bt}")
        nc.tensor.matmul(yb[:], gT[:], bias_sb[:], start=True, stop=True)
        ybs = ybspool.tile([P, DOUT], F32, tag="ybs", name=f"ybs{bt}")
        nc.scalar.copy(ybs[:], yb[:])
        acc = accpool.tile([P, DOUT], F32, tag="acc", name=f"acc{bt}")
        for j in range(G):
            y = matmul_y(xT, j)
            gcol = gB[:, k, j:j + 1]
            fused_step(acc, y, gcol, seed=ybs if j == 0 else None)
        nc.sync.dma_start(out_ap[bt * P:(bt + 1) * P, :], acc[:])


def _build():
    nc = bacc.Bacc("TRN2", target_bir_lowering=False, debug=False)
    # x/weight/bias declared float32r (same 4-byte layout as fp32 on the
    # host) so DMA feeds the FP32r matmuls/transposes with no conversion
    x_ap = nc.dram_tensor("x", [BC, DIN], F32R, kind="ExternalInput").ap()
    g_ap = nc.dram_tensor("g", [BC, G], F32, kind="ExternalInput").ap()
    w_ap = nc.dram_tensor("weight", [G, DIN, DOUT], F32R, kind="ExternalInput").ap()
    bias_ap = nc.dram_tensor("bias_p", [G, DOUT], F32R, kind="ExternalInput").ap()
    out_ap = nc.dram_tensor("out", [BC, DOUT], F32, kind="ExternalOutput").ap()

    from contextlib import ExitStack

    with tile.TileContext(nc) as tc:
        with ExitStack() as ctx:
            _emit(nc, tc, out_ap, x_ap, g_ap, w_ap, bias_ap, ctx)
    nc.compile()
    return nc


_NC = None
last_result = None


def kernel(x, g, weight, bias_p):
    global _NC, last_result
    if _NC is None:
        _NC = _build()

    x = np.ascontiguousarray(np.asarray(x, dtype=np.float32))
    g = np.ascontiguousarray(np.asarray(g, dtype=np.float32))
    weight = np.ascontiguousarray(np.asarray(weight, dtype=np.float32))
    bias_p = np.ascontiguousarray(np.asarray(bias_p, dtype=np.float32))

    in_maps = [
        {
            "x": x[c * BC:(c + 1) * BC],
            "g": g[c * BC:(c + 1) * BC],
            "weight": weight,
            "bias_p": bias_p,
        }
        for c in range(NCORES)
    ]
    res = run_bass_kernel_spmd(_NC, in_maps, core_ids=list(range(NCORES)))
    last_result = res
    return np.concatenate([r["out"] for r in res.results], axis=0)


# revision 19
# speedup vs baseline: 1.0949x; 1.0067x over previous
"""GroupLinear (soft MoE routing) Trainium2 Bass kernel.

Computes out[b,o] = sum_j g[b,j] * (x[b,:] @ W[j,:,:])[o] + (g @ bias_p)[b,o]
for B=16384, G=16, DIN=DOUT=512, fp32.

Sharding: data-parallel over batch across 8 NeuronCores (2048 rows/core);
weight + bias replicated.

Per-core schedule (PE-roofline oriented; the PE stream is the critical path
at ~231 ns per 512-row fp32r matmul, and any multi-us PE stall also drops
the HAM clock to 4/8 for tens of us — so every engine that gates the PE
must stay far ahead):
  - W streams on the SP HWDGE queue; x0/g/bias startup loads on the
    Activation queue so W[0] and x0 transfer concurrently. g tiles for a
    whole phase load as ONE batched DMA ([128, 8, 16]).
  - fp32 warmup matmuls cover the framework preamble -> W[0] arrival window
    and start the HAM clock ramp.
  - group-mix accumulation uses the fused VectorE scalar_tensor_tensor:
    acc = y * g[:,j] + acc (one op per (j,tile) instead of scale+add, with
    the j=0 op seeding from the bias matmul: acc = y*g0 + yb). VectorE is
    the only PSUM drain for y (8.5us per 16-group chain vs 15.1us of PE per
    tile); ScalarE only does the small transpose copies, so neither can
    back-pressure the PE's PSUM rotation.
  - phase A (tiles 0-7): group loop outermost, paced by W arrival; x
    transposes (fp32r identity, 1.5 cy/row) + gT + bias matmul emitted
    per-tile inside the j=0 sweep.
  - phase B (tiles 8-15): tile loop outermost (W resident); each tile's
    output DMA streams out as soon as its chain ends - no tail burst.
"""

import numpy as np

import concourse.bass as bass
import concourse.tile as tile
from concourse import bacc, mybir
from concourse.bass_utils import run_bass_kernel_spmd
from concourse.masks import make_identity

B, G, DIN, DOUT = 16384, 16, 512, 512
NCORES = 8
BC = B // NCORES          # rows per core
P = 128                   # partitions
NBT = BC // P             # batch tiles per core (16)
KC = DIN // P             # contraction chunks (4)
PB = 8                    # batch tiles per phase
NPH = NBT // PB           # phases (2)

F32 = mybir.dt.float32
F32R = mybir.dt.float32r
MULT = mybir.AluOpType.mult
ADD = mybir.AluOpType.add


def _emit(nc, tc, out_ap, x_ap, g_ap, w_ap, bias_ap, ctx):
    const_pool = ctx.enter_context(tc.tile_pool(name="const", bufs=1))
    wpool = ctx.enter_context(tc.tile_pool(name="wpool", bufs=1))
    xpool = ctx.enter_context(tc.tile_pool(name="xpool", bufs=6))
    gpool = ctx.enter_context(tc.tile_pool(name="gpool", bufs=2))
    xtpool = ctx.enter_context(tc.tile_pool(name="xtpool", bufs=PB + 1))
    gtpool = ctx.enter_context(tc.tile_pool(name="gtpool", bufs=PB + 1))
    accpool = ctx.enter_context(tc.tile_pool(name="accpool", bufs=PB + 1))
    ybspool = ctx.enter_context(tc.tile_pool(name="ybspool", bufs=3))
    ps_y = ctx.enter_context(tc.tile_pool(name="ps_y", bufs=4, space="PSUM"))
    ps_yb = ctx.enter_context(tc.tile_pool(name="ps_yb", bufs=2, space="PSUM"))
    ps_t = ctx.enter_context(tc.tile_pool(name="ps_t", bufs=2, space="PSUM"))

    # fp32r identity: transpose cost is keyed on the moving operand (the
    # identity); fp32r streams 1.5 cy/row vs 2.0 for fp32. Built as fp32
    # (gpsimd memset/affine_select reject f32r), rounded into an f32r tile
    # via ScalarE copy (satisfies the rounded-to-FP32r BIR check; 0/1 are
    # exact). A plain fp32 identity serves the fp32 g transposes.
    ident32 = const_pool.tile([P, P], F32, name="ident32")
    make_identity(nc, ident32)
    identr = const_pool.tile([P, P], F32R, name="identr")
    nc.scalar.copy(identr[:], ident32[:])

    # startup loads: Activation HWDGE queue (frees SP for the W stream).
    # ScalarE is drain-idle until the first transposes, ~2us after these.
    xpA = xpool.tile([P, DIN], F32R, tag="xt", name="xt0")
    nc.scalar.dma_start(xpA[:], x_ap[0:P, :])
    gA = gpool.tile([P, PB, G], F32, tag="g", name="gA")
    nc.scalar.dma_start(
        gA[:], g_ap[0:PB * P, :].rearrange("(k p) j -> p k j", p=P)
    )
    bias_sb = const_pool.tile([G, DOUT], F32R, name="bias_sb")
    nc.scalar.dma_start(bias_sb[:], bias_ap[:, :])

    # PE warmup: dependency-free matmuls covering preamble -> W[0] arrival,
    # ramping the HAM clock before the real stream begins.
    dum = const_pool.tile([P, DOUT], F32, name="dum")
    nc.gpsimd.memset(dum[:], 1.0)
    for wi in range(3):
        wps = ps_t.tile([P, DOUT], F32, tag="tps", name="wps")
        nc.tensor.matmul(wps[:], dum[:, 0:P], dum[:], start=True, stop=True)

    w_sb = wpool.tile([P, G * KC * DOUT], F32R, name="w_sb")

    def issue_w(j):
        for ic in range(KC):
            nc.sync.dma_start(
                w_sb[:, (j * KC + ic) * DOUT:(j * KC + ic + 1) * DOUT],
                w_ap[j, ic * P:(ic + 1) * P, :],
            )

    def issue_x(bt):
        xt = xpool.tile([P, DIN], F32R, tag="xt", name=f"xt{bt}")
        nc.sync.dma_start(xt[:], x_ap[bt * P:(bt + 1) * P, :])
        return xt

    # SP queue order tuned so everything lands just ahead of first use:
    # W0 (needed ~10.5us), then ALL phase-A x tiles (their transposes run
    # in the pre-sweep block — a late x tile there dips PE duty and trips
    # the HAM idle monitor, halving the clock for ~20us), then the W body.
    xts = {0: xpA}
    issue_w(0)
    for bt in range(1, PB):
        xts[bt] = issue_x(bt)
    for j in range(1, G):
        issue_w(j)
    for bt in range(PB, NBT):
        xts[bt] = issue_x(bt)
    gB = gpool.tile([P, PB, G], F32, tag="g", name="gB")
    nc.sync.dma_start(
        gB[:], g_ap[PB * P:2 * PB * P, :].rearrange("(k p) j -> p k j", p=P)
    )

    def issue_transpose_x(xt, bt):
        """PE transposes of x (fp32r); PSUM->SBUF copies on ScalarE."""
        xT = xtpool.tile([P, DIN], F32R, tag="xT", name=f"xT{bt}")
        for ic in range(KC):
            tps = ps_t.tile([P, P], F32R, tag="tps", name="tps")
            nc.tensor.transpose(tps[:], xt[:, ic * P:(ic + 1) * P], identr[:])
            nc.scalar.copy(xT[:, ic * P:(ic + 1) * P], tps[:])
        return xT

    def issue_gt(gsrc, bt):
        gps = ps_t.tile([G, P], F32, tag="tps", name="gps")
        nc.tensor.transpose(gps[:], gsrc, ident32[:])
        gT = gtpool.tile([G, P], F32R, tag="gT", name=f"gT{bt}")
        nc.scalar.copy(gT[:], gps[:])
        return gT

    def matmul_y(xT, j):
        y = ps_y.tile([P, DOUT], F32, tag="y", name="y")
        for ic in range(KC):
            nc.tensor.matmul(
                y[:],
                xT[:, ic * P:(ic + 1) * P],
                w_sb[:, (j * KC + ic) * DOUT:(j * KC + ic + 1) * DOUT],
                start=(ic == 0),
                stop=(ic == KC - 1),
            )
        return y

    def fused_step(acc, y, gcol, seed=None):
        # acc = y * g[:,j] + (yb at j=0 else acc): one VectorE op drains the
        # y PSUM bank, applies the group weight, and accumulates.
        nc.vector.scalar_tensor_tensor(
            acc[:], y[:], gcol, (seed if seed is not None else acc)[:], MULT, ADD
        )

    # ---- phase A: tiles 0..7, group loop outermost (paced by W arrival) ----
    # All transposes run as one block BEFORE the sweeps, while the HAM clock
    # is still cold: their dependency gaps are harmless at 4/8, and the
    # sweeps that follow are a dense 100%-duty matmul stream that keeps the
    # released 8/8 clock from re-throttling.
    trs = {}
    gts = {}
    for k in range(PB):
        trs[k] = issue_transpose_x(xts[k], k)
        gts[k] = issue_gt(gA[:, k, :], k)

    accs = {}
    for j in range(G):
        for k in range(PB):
            gcol = gA[:, k, j:j + 1]
            if j == 0:
                yb = ps_yb.tile([P, DOUT], F32, tag="yb", name=f"yb{k}")
                nc.tensor.matmul(yb[:], gts[k][:], bias_sb[:], start=True, stop=True)
                y = matmul_y(trs[k], j)
                # the fused op may read only one PSUM operand; stage the
                # bias term through SBUF on the (otherwise idle) ScalarE
                ybs = ybspool.tile([P, DOUT], F32, tag="ybs", name=f"ybs{k}")
                nc.scalar.copy(ybs[:], yb[:])
                acc = accpool.tile([P, DOUT], F32, tag="acc", name=f"acc{k}")
                accs[k] = acc
                fused_step(acc, y, gcol, seed=ybs)
            else:
                y = matmul_y(trs[k], j)
                fused_step(accs[k], y, gcol)

    for k in range(PB):
        nc.sync.dma_start(out_ap[k * P:(k + 1) * P, :], accs[k][:])

    # ---- phase B: tiles 8..15, tile loop outermost (W fully resident) ----
    for bt in range(PB, NBT):
        k = bt - PB
        xT = issue_transpose_x(xts[bt], bt)
        gT = issue_gt(gB[:, k, :], bt)
        yb = ps_yb.tile([P, DOUT], F32, tag="yb", name=f"yb{bt}")
        nc.tensor.matmul(yb[:], gT[:], bias_sb[:], start=True, stop=True)
        ybs = ybspool.tile([P, DOUT], F32, tag="ybs", name=f"ybs{bt}")
        nc.scalar.copy(ybs[:], yb[:])
        acc = accpool.tile([P, DOUT], F32, tag="acc", name=f"acc{bt}")
        for j in range(G):
            y = matmul_y(xT, j)
            gcol = gB[:, k, j:j + 1]
            fused_step(acc, y, gcol, seed=ybs if j == 0 else None)
        nc.sync.dma_start(out_ap[bt * P:(bt + 1) * P, :], acc[:])


def _build():
    nc = bacc.Bacc("TRN2", target_bir_lowering=False, debug=False)
    # x/weight/bias declared float32r (same 4-byte layout as fp32 on the
    # host) so DMA feeds the FP32r matmuls/transposes with no conversion
    x_ap = nc.dram_tensor("x", [BC, DIN], F32R, kind="ExternalInput").ap()
    g_ap = nc.dram_tensor("g", [BC, G], F32, kind="ExternalInput").ap()
    w_ap = nc.dram_tensor("weight", [G, DIN, DOUT], F32R, kind="ExternalInput").ap()
    bias_ap = nc.dram_tensor("bias_p", [G, DOUT], F32R, kind="ExternalInput").ap()
    out_ap = nc.dram_tensor("out", [BC, DOUT], F32, kind="ExternalOutput").ap()

    from contextlib import ExitStack

    with tile.TileContext(nc) as tc:
        with ExitStack() as ctx:
            _emit(nc, tc, out_ap, x_ap, g_ap, w_ap, bias_ap, ctx)
    nc.compile()
    return nc


_NC = None
last_result = None


def kernel(x, g, weight, bias_p):
    global _NC, last_result
    if _NC is None:
        _NC = _build()

    x = np.ascontiguousarray(np.asarray(x, dtype=np.float32))
    g = np.ascontiguousarray(np.asarray(g, dtype=np.float32))
    weight = np.ascontiguousarray(np.asarray(weight, dtype=np.float32))
    bias_p = np.ascontiguousarray(np.asarray(bias_p, dtype=np.float32))

    in_maps = [
        {
            "x": x[c * BC:(c + 1) * BC],
            "g": g[c * BC:(c + 1) * BC],
            "weight": weight,
            "bias_p": bias_p,
        }
        for c in range(NCORES)
    ]
    res = run_bass_kernel_spmd(_NC, in_maps, core_ids=list(range(NCORES)))
    last_result = res
    return np.concatenate([r["out"] for r in res.results], axis=0)
